# revision 31
# baseline (speedup 1.0000x reference)
"""Bahdanau-attention kernel for TRN2, data-parallel over batch on 8 NeuronCores.

Reference computation (B=64, S=1024, H=512):
    energy    = tanh(cat([hidden bcast S, enc], -1) @ attn_w.T + attn_b)  [B,S,H]
    attention = energy @ v_w.T                                            [B,S]
    out       = softmax(where(mask==0, -1e10, attention), axis=1)

Kernel strategy (per core, 8 batch rows):
  - c[b] = W_h @ hidden[b] + attn_b is computed once on-device ([8,512] via
    four K=128 matmuls from hiddenT), then folded into each energy tile's
    PSUM accumulation as a fifth K=8 matmul with lhsT = sel8t (one-hot of
    b = p mod 8), so energy[r,:] = W_e @ enc[r] + c[b(r)] lands in PSUM with
    no extra vector-engine pass. Main matmul runs in float32r (full PE rate),
    tanh on ACT straight from PSUM, fused v-multiply + free-axis reduction on
    DVE, and a max-free softmax (logits are bounded by |v|_1 < 12, so exp
    cannot overflow and the mask is applied as exp(att)*mask01).
  - Row order is s-major: r = s*8 + b_loc. Tile t covers s in [16t,16t+16).
    att_all[p,t] with p = 8*(s%16)+b_loc. Group sums use a sel8 matmul; the
    [8,1024] output layout is recovered with 16 identity-slice matmuls.
"""
import numpy as np

import concourse.bass as bass
import concourse.tile as tile
from concourse import bacc, mybir
from concourse.bass_utils import run_bass_kernel_spmd

B, S, H = 64, 1024, 512
NCORES = 8
BLOC = B // NCORES              # 8 batch rows per core
R = S * BLOC                    # 8192 rows per core
NT = R // 128                   # 64 tiles of 128 rows
TPB = 4                         # tiles per DMA block
NBLK = NT // TPB                # 16 blocks
SLO = 16                        # s-positions per tile per batch row
F32, F32R = mybir.dt.float32, mybir.dt.float32r
BF16 = mybir.dt.bfloat16
AF = mybir.ActivationFunctionType
ALU = mybir.AluOpType

_CACHE = {}


def _build():
    nc = bacc.Bacc(None)
    enc_t = nc.dram_tensor("enc_t", [H, R], F32, kind="ExternalInput")
    wet = nc.dram_tensor("wet", [H, H], F32, kind="ExternalInput")       # [j, h]
    wht = nc.dram_tensor("wht", [H, H], F32, kind="ExternalInput")       # [j, h]
    hid_t = nc.dram_tensor("hid_t", [128, (H // 128) * BLOC], F32, kind="ExternalInput")
    b1 = nc.dram_tensor("b1", [1, H], F32, kind="ExternalInput")
    v = nc.dram_tensor("v", [1, H], F32, kind="ExternalInput")
    mask01 = nc.dram_tensor("mask01", [128, NT], F32, kind="ExternalInput")
    ident = nc.dram_tensor("ident", [128, 128], BF16, kind="ExternalInput")
    sel8 = nc.dram_tensor("sel8", [128, BLOC], BF16, kind="ExternalInput")
    sel8t = nc.dram_tensor("sel8t", [BLOC, 128], BF16, kind="ExternalInput")
    out = nc.dram_tensor("out", [BLOC, S], F32, kind="ExternalOutput")

    njc = H // 128
    with tile.TileContext(nc) as tc:
        with tc.tile_pool(name="singles", bufs=1) as singles, \
             tc.tile_pool(name="enc", bufs=4) as encp, \
             tc.tile_pool(name="work", bufs=4) as work, \
             tc.tile_pool(name="ps", bufs=6, space="PSUM") as ps, \
             tc.tile_pool(name="ps1", bufs=1, space="PSUM") as ps1:

            enc_view = enc_t.ap().rearrange(
                "(c k) (blk r) -> k c blk r", k=128, r=TPB * 128)
            enc_sbs = []
            for _blk in range(NBLK):
                enc_sb = encp.tile([128, njc, TPB * 128], F32R, tag="enc")
                enc_sbs.append(enc_sb)

            # interleave wet / enc block0 / wht chunks: tile 0's matmuls need
            # wet+enc0 only, the hterm matmuls need wht shortly after
            wet_sb = singles.tile([128, njc, H], F32R, tag="wet")
            wht_sb = singles.tile([128, njc, H], F32R, tag="wht")
            for jc in range(njc):
                nc.sync.dma_start(
                    out=wet_sb[:, jc, :],
                    in_=wet[jc * 128:(jc + 1) * 128, :].bitcast(F32R),
                )
                nc.sync.dma_start(
                    out=enc_sbs[0][:, jc, :],
                    in_=enc_view[:, jc, 0, :].bitcast(F32R),
                )
                nc.sync.dma_start(
                    out=wht_sb[:, jc, :],
                    in_=wht[jc * 128:(jc + 1) * 128, :].bitcast(F32R),
                )
            hid_sb = singles.tile([128, njc, BLOC], F32R, tag="hid")
            nc.sync.dma_start(
                out=hid_sb,
                in_=hid_t.ap().rearrange("k (c b) -> k c b", c=njc).bitcast(F32R),
            )
            sel8t_sb = singles.tile([BLOC, 128], BF16, tag="sel8t")
            nc.sync.dma_start(out=sel8t_sb, in_=sel8t[:])
            b8_sb = singles.tile([BLOC, H], F32, tag="b8")
            nc.gpsimd.dma_start(out=b8_sb, in_=b1.ap().partition_broadcast(BLOC))
            v_sb = singles.tile([128, H], F32, tag="v")
            nc.gpsimd.dma_start(out=v_sb, in_=v.ap().partition_broadcast(128))

            # issue every remaining enc DMA up-front so the sync queue never
            # idles; the enc pool's WAR deps pace them against consumption
            for blk in range(1, NBLK):
                for jc in range(njc):
                    nc.sync.dma_start(
                        out=enc_sbs[blk][:, jc, :],
                        in_=enc_view[:, jc, blk, :].bitcast(F32R),
                    )

            # epilogue-only constants load behind the enc stream
            mask_sb = singles.tile([128, NT], F32, tag="mask")
            nc.gpsimd.dma_start(out=mask_sb, in_=mask01[:])
            ident_sb = singles.tile([128, 128], BF16, tag="ident")
            nc.gpsimd.dma_start(out=ident_sb, in_=ident[:])
            sel8_sb = singles.tile([128, BLOC], BF16, tag="sel8")
            nc.gpsimd.dma_start(out=sel8_sb, in_=sel8[:])

            att_all = singles.tile([128, NT], F32, tag="att")
            dummy = singles.tile([128, 1], F32, tag="dummy")

            def enc_group(blk, tl):
                psum_e = ps.tile([128, H], F32, tag="pe")
                for jc in range(njc):
                    nc.tensor.matmul(
                        psum_e,
                        enc_sbs[blk][:, jc, tl * 128:(tl + 1) * 128],
                        wet_sb[:, jc, :],
                        start=(jc == 0),
                        stop=False,
                    )
                return psum_e

            def finish_tile(t, psum_e, c_rep):
                nc.tensor.matmul(psum_e, ident_sb, c_rep,
                                 start=False, stop=True)
                tanh_sb = work.tile([128, H], F32, tag="tanh")
                nc.scalar.activation(tanh_sb, psum_e, AF.Tanh)
                nc.vector.scalar_tensor_tensor(
                    out=dummy.broadcast_to([128, H]),
                    in0=tanh_sb, scalar=0.0, in1=v_sb,
                    op0=ALU.bypass, op1=ALU.mult,
                    accum_out=att_all[:, t:t + 1],
                )

            # block 0: run all four jc-groups first (the 4 psum bufs hold
            # them), then the hterm chain, then the folds — so the wht wait
            # hides under tile 0-3 matmuls
            blk0_psums = [enc_group(0, tl) for tl in range(TPB)]
            psum_c = ps1.tile([BLOC, H], F32, tag="pc")
            for jc in range(njc):
                nc.tensor.matmul(
                    psum_c, hid_sb[:, jc, :], wht_sb[:, jc, :],
                    start=(jc == 0), stop=(jc == njc - 1),
                )
            c_sb = singles.tile([BLOC, H], BF16, tag="c8")
            nc.vector.scalar_tensor_tensor(
                out=c_sb, in0=psum_c, scalar=0.0, in1=b8_sb,
                op0=ALU.bypass, op1=ALU.add,
            )
            # replicate c to all 128 partitions (row p = c[p mod 8]) so the
            # per-tile fold streams a full-partition moving operand
            psum_cr = ps1.tile([128, H], F32, tag="pc")
            nc.tensor.matmul(psum_cr, sel8t_sb, c_sb, start=True, stop=True)
            c_rep = singles.tile([128, H], BF16, tag="crep")
            nc.scalar.copy(out=c_rep, in_=psum_cr)
            for tl in range(TPB):
                finish_tile(tl, blk0_psums[tl], c_rep)

            for blk in range(1, NBLK):
                for tl in range(TPB):
                    psum_e = enc_group(blk, tl)
                    finish_tile(blk * TPB + tl, psum_e, c_rep)

            # --- softmax epilogue (no max subtraction: |att| <= |v|_1 < 12) ---
            e_all = singles.tile([128, NT], F32, tag="e_all")
            nc.scalar.activation(e_all, att_all, AF.Exp)
            em = singles.tile([128, NT], BF16, tag="em")
            nc.vector.tensor_tensor(out=em, in0=e_all, in1=mask_sb, op=ALU.mult)

            psum_d = ps1.tile([BLOC, NT], F32, tag="pc")
            nc.tensor.matmul(psum_d, sel8_sb, em, start=True, stop=True)
            den8 = singles.tile([BLOC, 1], F32, tag="den8")
            nc.vector.tensor_reduce(den8, psum_d, mybir.AxisListType.X, ALU.add)
            r8 = singles.tile([BLOC, 1], F32, tag="r8")
            nc.vector.reciprocal(r8, den8)

            psum_o = ps1.tile([BLOC, S], F32, tag="pc")
            for sl in range(SLO):
                nc.tensor.matmul(
                    psum_o[:, sl * NT:(sl + 1) * NT],
                    ident_sb[:, sl * BLOC:(sl + 1) * BLOC],
                    em,
                    start=True, stop=True,
                )
            out_sb = singles.tile([BLOC, S], F32, tag="out")
            nc.vector.tensor_scalar(
                out=out_sb,
                in0=psum_o.rearrange("p (sl t) -> p t sl", sl=SLO),
                scalar1=r8, scalar2=None, op0=ALU.mult,
            )
            nc.sync.dma_start(out=out[:], in_=out_sb)
    nc.finalize()
    return nc


def _prep(hidden, encoder_outputs, attn_mask, attn_w, attn_b, v_w):
    """Host-side shard prep. Returns in_maps for the 8 cores."""
    hidden = np.asarray(hidden, np.float32)
    enc = np.asarray(encoder_outputs, np.float32)        # [S, B, H]
    mask = np.asarray(attn_mask)
    attn_w = np.asarray(attn_w, np.float32)              # [H, 2H]
    attn_b = np.asarray(attn_b, np.float32)
    v_w = np.asarray(v_w, np.float32).reshape(1, H)

    wet = np.ascontiguousarray(attn_w[:, H:].T)          # [j, h]
    wht = np.ascontiguousarray(attn_w[:, :H].T)          # [j, h]
    b1 = np.ascontiguousarray(attn_b.reshape(1, H))
    import ml_dtypes
    ident = np.eye(128).astype(ml_dtypes.bfloat16)
    sel8 = np.tile(np.eye(BLOC), (SLO, 1)).astype(ml_dtypes.bfloat16)  # [128, 8]
    sel8t = np.ascontiguousarray(sel8.T).astype(ml_dtypes.bfloat16)  # [8, 128]

    in_maps = []
    for core in range(NCORES):
        bsl = slice(core * BLOC, (core + 1) * BLOC)
        shard = enc[:, bsl, :]                           # [S, 8, H]
        enc_t = np.ascontiguousarray(
            shard.reshape(R, H).T)                       # [H, R], r = s*8+b
        # hid_t[k, c, b] = hidden[b, c*128+k], flattened to [128, 4*8]
        hid_t = np.ascontiguousarray(
            hidden[bsl].T.reshape(H // 128, 128, BLOC)
            .transpose(1, 0, 2).reshape(128, (H // 128) * BLOC))
        m = mask[bsl, :]                                 # [8, S]
        # mask01[p, t] = mask[b, 16t + s_lo], p = 8*s_lo + b
        m01 = np.ascontiguousarray(
            (m != 0).astype(np.float32)                  # [8, S]
            .reshape(BLOC, NT, SLO)                      # s = 16t + s_lo
            .transpose(2, 0, 1)                          # [s_lo, b, t]
            .reshape(128, NT))
        in_maps.append({
            "enc_t": enc_t, "wet": wet, "wht": wht, "hid_t": hid_t,
            "b1": b1, "v": v_w, "mask01": m01,
            "ident": ident, "sel8": sel8, "sel8t": sel8t,
        })
    return in_maps


def kernel(t, hidden, encoder_outputs, attn_mask, src_gps_seqs, src,
           src_rids, input_id, trg_gps_seqs, attn_w, attn_b, v_w):
    if "nc" not in _CACHE:
        _CACHE["nc"] = _build()
    nc = _CACHE["nc"]
    in_maps = _prep(hidden, encoder_outputs, attn_mask, attn_w, attn_b, v_w)
    res = run_bass_kernel_spmd(nc, in_maps, core_ids=list(range(NCORES)))
    out = np.empty((B, S), np.float32)
    for core in range(NCORES):
        out[core * BLOC:(core + 1) * BLOC] = res.results[core]["out"]
    return out
